# revision 11
# baseline (speedup 1.0000x reference)
"""Causal attention (B=4096, T=64, C=64) on 8 TRN2 NeuronCores, pure data parallel.

Per core: x shard [512, 64, 64]. 512-token macro-tiles (8 batches), bf16 matmul
operands (f32 PSUM accumulate), 2-way tile_position packing: even batches live on
partitions 0-63, odd batches on partitions 64-127, so per-batch matmuls run as
concurrent pairs in the two halves of the PE array.

Per tile:
  x loaded permuted (partition p <- tokens 4p..4p+3, 1KB contiguous descriptors);
  PE transposes write strided psum columns to restore natural token order.
  xT = transpose(x)                     [64, 512] bf16
  qT2/kT2[p, j, t]: rows 0:64 = even batches, 64:128 = odd  (strided rhs views)
  v[p, j, c] = xT_chunk.T @ WvT         (token-chunk layout == parity split)
  weiT psum = causal mask (identity matmul) + kT_b.T @ qT_b (accumulated);
  weiT_e = exp(0.125 * psum) -> bf16    (ACT reads PSUM directly)
  sums[p, j] = weiT_e.T @ ones ; recip = 1/sums   (matches y chunk layout)
  outT[c half, j, t] = v_b.T @ weiT_e
  y[p, j, c] = outT_chunk.T @ WpT ; y = y*recip + bp ; DMA out
"""

import numpy as np

import concourse.bass as bass
import concourse.mybir as mybir
import concourse.tile as tile
import concourse.masks as masks
from concourse import bacc

F32 = mybir.dt.float32
BF16 = mybir.dt.bfloat16

N_CORES = 8
B, T, C = 4096, 64, 64
B_LOC = B // N_CORES  # 512 batches per core

MASK_VAL = -1e9


def build_nc(b_loc=B_LOC, batches_per_tile=8, reps=1):
    """Build the single-core Bass graph (SPMD: same graph on all 8 cores)."""
    assert b_loc % batches_per_tile == 0
    n_tiles = b_loc // batches_per_tile
    TOK = batches_per_tile * T              # tokens per macro tile (512)
    NCH = TOK // 128                        # 128-token chunks per tile (4)

    nc = bacc.Bacc("TRN2", target_bir_lowering=False, debug=False)

    x_ext = nc.declare_dram_parameter("x", [b_loc, T, C], F32, isOutput=False)
    Wk_ext = nc.declare_dram_parameter("Wk", [C, C], F32, isOutput=False)
    Wq_ext = nc.declare_dram_parameter("Wq", [C, C], F32, isOutput=False)
    Wv_ext = nc.declare_dram_parameter("Wv", [C, C], F32, isOutput=False)
    Wp_ext = nc.declare_dram_parameter("Wp", [C, C], F32, isOutput=False)
    bp_ext = nc.declare_dram_parameter("bp", [C], F32, isOutput=False)
    out_ext = nc.declare_dram_parameter("out", [b_loc, T, C], F32, isOutput=True)

    x_flat = x_ext.ap().rearrange("b t c -> (b t) c")
    out_flat = out_ext.ap().rearrange("b t c -> (b t) c")

    # maskT[s, t] = 0 where s <= t else MASK_VAL*8 (exp scale 0.125 applied after)
    m1 = np.where(
        np.arange(T)[:, None] <= np.arange(T)[None, :], 0.0, MASK_VAL * 8.0
    ).astype(np.float32)
    maskT2_dram = nc.inline_tensor(np.vstack([m1, m1]), name="maskT2_const")
    ident_dram = nc.inline_tensor(np.eye(128, dtype=np.float32), name="ident_const")

    with tile.TileContext(nc) as tc:
        with (
            tc.tile_pool(name="const", bufs=1) as constp,
            tc.tile_pool(name="xin", bufs=3) as xin_pool,
            tc.tile_pool(name="work", bufs=3) as work_pool,
            tc.tile_pool(name="yout", bufs=3) as yout_pool,
            tc.tile_pool(name="ps", bufs=2, space="PSUM") as ps,
        ):
            # ---- one-time constants ----
            ident = constp.tile([128, 128], F32)
            nc.sync.dma_start(ident[:], ident_dram.ap())
            maskT2f = constp.tile([128, T], F32)
            nc.sync.dma_start(maskT2f[:], maskT2_dram.ap())
            ident_bf = constp.tile([128, 128], BF16)
            masks.make_identity(nc, ident_bf[:])
            maskT2 = constp.tile([128, T], BF16)
            nc.vector.tensor_copy(maskT2[:], maskT2f[:])
            ones128 = constp.tile([128, 1], BF16)
            nc.vector.memset(ones128[:], 1.0)

            # weights: natural DMA (contiguous), PE transpose, cast to bf16
            wnat = constp.tile([C, 4 * C], F32)
            for i, w_ext in enumerate((Wq_ext, Wk_ext, Wv_ext, Wp_ext)):
                nc.sync.dma_start(wnat[:, i * C : (i + 1) * C], w_ext.ap())
            wT_ps = ps.tile([C, 4 * C], F32, tag="A")
            for i in range(4):
                nc.tensor.transpose(
                    wT_ps[:, i * C : (i + 1) * C],
                    wnat[:, i * C : (i + 1) * C],
                    ident[0:C, 0:C],
                )
            WqT = constp.tile([C, C], BF16)
            nc.vector.tensor_copy(WqT[:], wT_ps[:, 0 * C : 1 * C])
            WkT = constp.tile([C, C], BF16)
            nc.vector.tensor_copy(WkT[:], wT_ps[:, 1 * C : 2 * C])
            WvT = constp.tile([C, C], BF16)
            nc.vector.tensor_copy(WvT[:], wT_ps[:, 2 * C : 3 * C])
            WpT128 = constp.tile([128, C], BF16)
            nc.vector.tensor_copy(WpT128[0:T, :], wT_ps[:, 3 * C : 4 * C])
            nc.vector.tensor_copy(WpT128[T:128, :], wT_ps[:, 3 * C : 4 * C])

            # bias broadcast to [128, C] via ones-matmul (K=1)
            bp_row = constp.tile([1, C], F32)
            nc.sync.dma_start(bp_row[:], bp_ext.ap().unsqueeze(0))
            ones_row128 = constp.tile([1, 128], F32)
            nc.vector.memset(ones_row128[:], 1.0)
            bias_ps = ps.tile([128, C], F32, tag="D")
            nc.tensor.matmul(bias_ps[:], ones_row128[:], bp_row[:])
            bias_bc = constp.tile([128, C], F32)
            nc.vector.tensor_copy(bias_bc[:], bias_ps[:])

            rep_ctx = tc.For_i(0, reps, 1) if reps > 1 else None
            if rep_ctx is not None:
                rep_ctx.__enter__()
            for it in range(n_tiles):
                t0 = it * TOK

                # ---- load x tile, permuted: partition p <- tokens 4p..4p+3 ----
                x_sb = xin_pool.tile([128, 4 * C], F32, tag="x_sb")
                nc.sync.dma_start(
                    x_sb[:],
                    x_flat[t0 : t0 + TOK, :].rearrange("(p m) c -> p (m c)", m=4),
                )

                # ---- transpose -> xT [64, TOK] in natural token order ----
                xT_ps = ps.tile([C, TOK], F32, tag="A")
                for m in range(4):
                    nc.tensor.transpose(
                        xT_ps[:, m * 128 : (m + 1) * 128],
                        x_sb[:, m * C : (m + 1) * C],
                        ident[:],
                    )
                # psum col m*128+p holds token 4p+m; un-permute in the copy
                xT = work_pool.tile([C, TOK], BF16, tag="xT")
                nc.scalar.copy(
                    xT[:].rearrange("c (p m) -> c m p", m=4),
                    xT_ps[:].rearrange("c (m p) -> c m p", p=128),
                )
                # even/odd batch column views of xT (strided)
                xT_eo = xT[:].rearrange("c (j e t) -> c j e t", e=2, t=T)
                xT_ev = xT_eo[:, :, 0, :]
                xT_od = xT_eo[:, :, 1, :]

                # ---- q/k projections directly into parity-split layout ----
                qT_ps = ps.tile([128, NCH, T], F32, tag="B")
                nc.tensor.matmul(qT_ps[0:T, :, :], WqT[:], xT_ev)
                nc.tensor.matmul(
                    qT_ps[T:128, :, :], WqT[:], xT_od, tile_position=(0, 64)
                )
                qT2 = work_pool.tile([128, NCH, T], BF16, tag="qT2")
                nc.scalar.copy(qT2[:], qT_ps[:])

                kT_ps = ps.tile([128, NCH, T], F32, tag="B")
                nc.tensor.matmul(kT_ps[0:T, :, :], WkT[:], xT_ev)
                nc.tensor.matmul(
                    kT_ps[T:128, :, :], WkT[:], xT_od, tile_position=(0, 64)
                )
                kT2 = work_pool.tile([128, NCH, T], BF16, tag="kT2")
                nc.vector.tensor_copy(kT2[:], kT_ps[:])

                # ---- v in natural token-chunk layout (== parity split) ----
                v_ps = ps.tile([128, NCH, C], F32, tag="C")
                for j in range(NCH):
                    nc.tensor.matmul(
                        v_ps[:, j, :], xT[:, j * 128 : (j + 1) * 128], WvT[:]
                    )
                v_sb = work_pool.tile([128, NCH, C], BF16, tag="v_sb")
                nc.vector.tensor_copy(v_sb[:], v_ps[:])

                # ---- scores: psum = causal mask, += kT_b.T @ qT_b (pairs) ----
                weiT_ps = ps.tile([128, NCH, T], F32, tag="D")
                for j in range(NCH):
                    nc.tensor.matmul(
                        weiT_ps[0:T, j, :], ident_bf[0:T, 0:T], maskT2[0:T, :],
                        start=True, stop=False,
                    )
                    nc.tensor.matmul(
                        weiT_ps[0:T, j, :], kT2[0:T, j, :], qT2[0:T, j, :],
                        start=False, stop=True,
                    )
                    nc.tensor.matmul(
                        weiT_ps[T:128, j, :],
                        ident_bf[T:128, T:128], maskT2[T:128, :],
                        start=True, stop=False, tile_position=(64, 64),
                    )
                    nc.tensor.matmul(
                        weiT_ps[T:128, j, :],
                        kT2[T:128, j, :], qT2[T:128, j, :],
                        start=False, stop=True, tile_position=(64, 64),
                    )
                weiT_e = work_pool.tile([128, NCH, T], BF16, tag="weiT_e")
                nc.scalar.activation(
                    weiT_e[:], weiT_ps[:], mybir.ActivationFunctionType.Exp,
                    scale=0.125,
                )

                # ---- softmax denominators; layout matches y chunks ----
                sums_ps = ps.tile([128, NCH], F32, tag="D")
                for j in range(NCH):
                    nc.tensor.matmul(
                        sums_ps[0:T, j : j + 1], weiT_e[0:T, j, :], ones128[0:T, :]
                    )
                    nc.tensor.matmul(
                        sums_ps[T:128, j : j + 1],
                        weiT_e[T:128, j, :],
                        ones128[T:128, :],
                        tile_position=(64, 64),
                    )
                recip = work_pool.tile([128, NCH], F32, tag="recip")
                nc.vector.reciprocal(recip[:], sums_ps[:])

                # ---- att @ v (transposed), packed pairs ----
                outT_ps = ps.tile([128, NCH, T], F32, tag="A")
                for j in range(NCH):
                    nc.tensor.matmul(
                        outT_ps[0:T, j, :], v_sb[0:T, j, :], weiT_e[0:T, j, :]
                    )
                    nc.tensor.matmul(
                        outT_ps[T:128, j, :],
                        v_sb[T:128, j, :],
                        weiT_e[T:128, j, :],
                        tile_position=(64, 64),
                    )
                outT2 = work_pool.tile([128, NCH, T], BF16, tag="outT2")
                nc.scalar.copy(outT2[:], outT_ps[:])

                # ---- output projection, packed pairs ----
                y_ps = ps.tile([128, NCH, C], F32, tag="C")
                for j in range(NCH):
                    nc.tensor.matmul(
                        y_ps[0:T, j, :], outT2[0:T, j, :], WpT128[0:T, :]
                    )
                    nc.tensor.matmul(
                        y_ps[T:128, j, :],
                        outT2[T:128, j, :],
                        WpT128[T:128, :],
                        tile_position=(64, 64),
                    )
                # ---- y = y*recip + bias ; store ----
                y_sb = yout_pool.tile([128, NCH, C], F32, tag="y_sb")
                for j in range(NCH):
                    nc.vector.scalar_tensor_tensor(
                        y_sb[:, j, :],
                        y_ps[:, j, :],
                        recip[:, j : j + 1],
                        bias_bc[:],
                        mybir.AluOpType.mult,
                        mybir.AluOpType.add,
                    )
                nc.sync.dma_start(
                    out_flat[t0 : t0 + TOK, :].rearrange("(j p) c -> p j c", p=128),
                    y_sb[:],
                )
            if rep_ctx is not None:
                rep_ctx.__exit__(None, None, None)

    nc.compile()
    return nc


_NC_CACHE = {}


def _get_nc(b_loc, batches_per_tile=8):
    key = (b_loc, batches_per_tile)
    if key not in _NC_CACHE:
        _NC_CACHE[key] = build_nc(b_loc, batches_per_tile)
    return _NC_CACHE[key]


def kernel(x, Wk, Wq, Wv, Wp, bp):
    from concourse.bass_utils import run_bass_kernel_spmd

    x = np.ascontiguousarray(x, dtype=np.float32)
    weights = {
        "Wk": np.ascontiguousarray(Wk, dtype=np.float32),
        "Wq": np.ascontiguousarray(Wq, dtype=np.float32),
        "Wv": np.ascontiguousarray(Wv, dtype=np.float32),
        "Wp": np.ascontiguousarray(Wp, dtype=np.float32),
        "bp": np.ascontiguousarray(bp, dtype=np.float32),
    }
    nc = _get_nc(B_LOC)
    in_maps = [
        {"x": x[i * B_LOC : (i + 1) * B_LOC], **weights} for i in range(N_CORES)
    ]
    res = run_bass_kernel_spmd(nc, in_maps, core_ids=list(range(N_CORES)))
    outs = [res.results[i]["out"] for i in range(N_CORES)]
    return np.concatenate(outs, axis=0)
